# revision 21
# baseline (speedup 1.0000x reference)
"""Trainium2 Bass kernel for nn_AdjacencyMatrix — v3: raw engine blocks +
XOR-addressed remote-DMA exchanges (no ncfw collectives).

Math: state_k = W * c_k[:,None] with c_{k+1} = W^T c_k, so the whole
module is num_steps chained matvecs; only the last 256 entries of c_4
(times diag(W)) are needed.  Column-parallel: core r owns W[:, 1024r:
1024(r+1)] (bf16, SBUF-resident) and produces the matching 1024-chunk of
each c_k.

The per-step chunk exchange is done with SBUF->SBUF remote_dma_broadcast
(one slot (0,k) per instruction -> peer tpb = own^k), not ncfw
collectives: this avoids the ~42us CC-stream barrier + ~11us cold-start +
HBM bounce that dominated the collective version.  XOR addressing means
receiver r's u-column block k holds sender r^k's chunk, so the host
permutes each core's SBUF W-panel order to match (position block k =
global k-tile block r^k).  Step 1 (x is only 1024 long) uses a separate
un-permuted 2MB W block; step 4 partials are exchanged the same way and
reduced locally.

All synchronization is manual semaphores (raw nc.Block, no TileContext —
Tile's single-core scheduling sim cannot model remotely-incremented
semaphores).
"""

import contextlib

import ml_dtypes
import numpy as np

import concourse.bass as bass
import concourse.mybir as mybir
from concourse import bacc
from concourse.bass_utils import run_bass_kernel_spmd

N = 8192
IN_N = 1024
OUT_N = 256
NCORES = 8
CP = N // NCORES
KT = N // 128
D0 = N - OUT_N
K0 = 8            # k-tiles in the step-1 (un-permuted) W block
NBLK = 4          # W chase blocks, 16 positions each
KPB = KT // NBLK

F32 = mybir.dt.float32
BF16 = mybir.dt.bfloat16

_cache: dict = {}


def _build(num_steps: int, debug: bool = False):
    assert num_steps >= 2
    n_mid = num_steps - 2
    nc = bacc.Bacc(
        "TRN2", target_bir_lowering=False, debug=debug, num_devices=NCORES
    )
    xT = nc.declare_dram_parameter("xT", [128, 8], BF16, isOutput=False)
    Wa = nc.declare_dram_parameter("Wa", [128, K0 * CP], BF16, isOutput=False)
    Wb = nc.declare_dram_parameter("Wb", [NBLK, 128, KPB * CP], BF16, isOutput=False)
    W4 = nc.declare_dram_parameter("W4", [128, 8 * OUT_N], BF16, isOutput=False)
    outp = nc.declare_dram_parameter("out", [128, 2], F32, isOutput=True)
    udump = nc.declare_dram_parameter("udump", [128, 64], BF16, isOutput=True)

    es = contextlib.ExitStack()
    with es:
        sem = {}
        for s in ["SX", "SPE", "SD", "SA", "SL", "SPR", "SG", "SO",
                  "SWA", "SW4", "SR"]:
            sem[s] = es.enter_context(nc.semaphore(s))
        SWB = [es.enter_context(nc.semaphore(f"SWB{b}")) for b in range(NBLK)]
        SE = [es.enter_context(nc.semaphore(f"SE{m}")) for m in range(n_mid + 1)]

        sb = lambda nm, sh, dt: es.enter_context(nc.sbuf_tensor(nm, sh, dt))
        ps = lambda nm, sh, dt: es.enter_context(nc.psum_tensor(nm, sh, dt))

        xt = sb("xt", [128, 8], BF16)
        wa = sb("wa", [128, K0 * CP], BF16)
        wk = sb("wk", [128, KT * CP], BF16)
        w4 = sb("w4", [128, 8 * OUT_N], BF16)
        onef = sb("onef", [1, 1], F32)
        ssb = [sb(f"s{m+1}sb", [1, 1024], F32) for m in range(n_mid + 1)]
        cT = [sb(f"c{m+1}T", [128, 8], BF16) for m in range(n_mid + 1)]
        u = [sb(f"u{m+2}", [128, 64], BF16) for m in range(n_mid)]
        u4 = sb("u4", [128, 8], BF16)
        s4sb = sb("s4sb", [1, OUT_N], F32)
        c4T = sb("c4T", [128, 2], F32)
        acc4 = sb("acc4", [128, 16], F32)
        res = sb("res", [128, 2], F32)
        rtmp = sb("rtmp", [128, 8], F32)

        pA = ps("pA", [128, 512], F32)
        pB = [ps(f"pB{m}", [128, 512], F32) for m in range(n_mid)]
        pD = ps("pD", [128, 512], F32)
        pT = ps("pT", [128, 8], F32)
        pT4 = ps("pT4", [128, 2], F32)

        # ---------------- sync: the W stream ----------------
        def prog_sync(sy):
            sy.dma_start(out=wa[:, :], in_=Wa.ap()).then_inc(sem["SWA"], 16)
            for b in range(NBLK):
                c0 = b * KPB * CP
                sy.dma_start(
                    out=wk[:, c0:c0 + KPB * CP], in_=Wb.ap()[b]
                ).then_inc(SWB[b], 16)
            sy.dma_start(out=w4[:, :], in_=W4.ap()).then_inc(sem["SW4"], 16)

        # ---------------- scalar: xt + step-4 evac ----------------
        def prog_scalar(sc):
            sc.dma_start(out=xt[:, :], in_=xT.ap()).then_inc(sem["SX"], 16)
            sc.wait_ge(sem["SPE"], 3 + 2 * n_mid)
            sc.copy(out=s4sb[0:1, :], in_=pD[0:1, 0:OUT_N]).then_inc(sem["SA"], 1)

        # ---------------- tensor: all matvecs + transposes ----------------
        def waves(pe, pout, usb, wsb, nk, chase=False):
            for p in range(nk):
                if chase and p % KPB == 0:
                    pe.wait_ge(SWB[p // KPB], 16)
                for g in range(4):
                    i = pe.matmul(
                        pout[32 * g:32 * g + 1, 0:256],
                        lhsT=usb[:, p:p + 1],
                        rhs=wsb[:, p * CP + 256 * g:p * CP + 256 * (g + 1)],
                        start=(p == 0),
                        stop=(p == nk - 1),
                        tile_position=(0, 32 * g),
                    )
            return i

        def prog_pe(pe):
            pe.wait_ge(sem["SX"], 16)
            pe.wait_ge(sem["SWA"], 16)
            # step 1 over wa; xt columns are contiguous (stride-1)
            for p in range(K0):
                for g in range(4):
                    i = pe.matmul(
                        pA[32 * g:32 * g + 1, 0:256],
                        lhsT=xt[:, p:p + 1],
                        rhs=wa[:, p * CP + 256 * g:p * CP + 256 * (g + 1)],
                        start=(p == 0),
                        stop=(p == K0 - 1),
                        tile_position=(0, 32 * g),
                    )
            i.then_inc(sem["SPE"], 1)                    # SPE 1: s1 psum done
            spe = 1
            for m in range(n_mid + 1):
                # chunk transposes: s{m+1}sb -> pT (all 4 evac copies done)
                pe.wait_ge(sem["SD"], 5 * m + 4)
                for kl in range(8):
                    i = pe.transpose(
                        pT[0:128, kl:kl + 1],
                        ssb[m][0:1, 128 * kl:128 * (kl + 1)],
                        onef[0:1, 0:1],
                    )
                i.then_inc(sem["SPE"], 1)                # SPE: transposes done
                spe += 1
                if m < n_mid:
                    pe.wait_ge(sem["SG"], m + 1)         # u[m] published
                    i = waves(pe, pB[m], u[m], wk, KT, chase=(m == 0))
                    i.then_inc(sem["SPE"], 1)            # SPE: mid matvec done
                    spe += 1
            # step 4: local tail-column matvec
            pe.wait_ge(sem["SD"], 5 * (n_mid + 1))       # u4 in SBUF
            pe.wait_ge(sem["SW4"], 16)                   # w4 loaded
            for kl in range(8):
                i = pe.matmul(
                    pD[0:1, 0:OUT_N],
                    lhsT=u4[:, kl:kl + 1],
                    rhs=w4[:, OUT_N * kl:OUT_N * (kl + 1)],
                    start=(kl == 0),
                    stop=(kl == 7),
                )
            i.then_inc(sem["SPE"], 1)                    # SPE 3+2n: s4 psum done
            pe.wait_ge(sem["SA"], 1)                     # s4sb evac'd
            for c in range(2):
                i = pe.transpose(
                    pT4[0:128, c:c + 1],
                    s4sb[0:1, 128 * c:128 * (c + 1)],
                    onef[0:1, 0:1],
                )
            i.then_inc(sem["SPE"], 1)                    # SPE 4+2n: pT4 done

        # ---------------- vector: evacs + psum->sbuf copies + reduce ------
        def prog_dve(ve):
            ve.memset(onef[0:1, 0:1], 1.0)
            for m in range(n_mid + 1):
                # evac psum matvec -> s{m+1}sb (4 disjoint copies; DVE
                # completion is out-of-order, so each gets its own inc)
                src_ = pA if m == 0 else pB[m - 1]
                ve.wait_ge(sem["SPE"], 1 + 2 * m)
                for g in range(4):
                    ve.tensor_copy(
                        ssb[m][0:1, 256 * g:256 * (g + 1)],
                        src_[32 * g:32 * g + 1, 0:256],
                    ).then_inc(sem["SD"], 1)
                # chunk copy pT -> c{m+1}T (or u4 for the last)
                ve.wait_ge(sem["SPE"], 2 + 2 * m)
                dst = cT[m] if m < n_mid else u4
                ve.tensor_copy(dst[:, 0:8], pT[0:128, 0:8]).then_inc(sem["SD"], 1)
            ve.wait_ge(sem["SPE"], 4 + 2 * n_mid)
            ve.tensor_copy(c4T[:, 0:2], pT4[0:128, 0:2]).then_inc(sem["SD"], 1)
            # final reduce: res = sum over the 8 slots of acc4 (sem-synced
            # tree; DVE has no intra-engine RAW interlock)
            ve.wait_ge(sem["SG"], n_mid + 1)
            for j in range(4):
                ve.scalar_tensor_tensor(
                    out=rtmp[:, 2 * j:2 * j + 2],
                    in0=acc4[:, 4 * j:4 * j + 2], scalar=0.0,
                    in1=acc4[:, 4 * j + 2:4 * j + 4],
                    op0=mybir.AluOpType.add, op1=mybir.AluOpType.add,
                ).then_inc(sem["SR"], 1)
            ve.wait_ge(sem["SR"], 4)
            for j in range(2):
                ve.scalar_tensor_tensor(
                    out=rtmp[:, 4 * j:4 * j + 2],
                    in0=rtmp[:, 2 * (2 * j):2 * (2 * j) + 2], scalar=0.0,
                    in1=rtmp[:, 2 * (2 * j + 1):2 * (2 * j + 1) + 2],
                    op0=mybir.AluOpType.add, op1=mybir.AluOpType.add,
                ).then_inc(sem["SR"], 1)
            ve.wait_ge(sem["SR"], 6)
            ve.scalar_tensor_tensor(
                out=res[:, 0:2], in0=rtmp[:, 0:2], scalar=0.0,
                in1=rtmp[:, 4:6],
                op0=mybir.AluOpType.add, op1=mybir.AluOpType.add,
            ).then_inc(sem["SD"], 1)

        # ---------------- gpsimd: exchanges + output ----------------
        def prog_pl(pl):
            spr = 0
            for m in range(n_mid + 1):
                last = m == n_mid
                src = c4T if last else cT[m]
                nbytes = 2 if last else 8
                dst = acc4 if last else u[m]
                pl.wait_ge(sem["SD"], 5 * (n_mid + 1) + 1 if last else 5 * m + 5)
                for k in range(1, NCORES):
                    rdests = [None] * NCORES
                    rdests[k] = (0, k)
                    pl.remote_dma_broadcast(
                        out_ap=dst[:, nbytes * k:nbytes * (k + 1)],
                        in_ap=src[:, 0:nbytes],
                        remote_sem=SE[m],
                        local_sem=sem["SL"],
                        rdests=rdests,
                    ).then_inc(sem["SPR"], 1)
                spr += 7
                pl.wait_ge(sem["SPR"], spr)
                pl.trigger_dma(count=7)
                pl.wait_ge(SE[m], 14)
                # self chunk; @complete inc publishes the whole u tile
                pl.tensor_copy(dst[:, 0:nbytes], src[:, 0:nbytes]).then_inc(
                    sem["SG"], 1
                )
            pl.wait_ge(sem["SD"], 5 * (n_mid + 1) + 2)
            pl.dma_start(out=outp.ap(), in_=res[:, 0:2]).then_inc(sem["SO"], 16)
            pl.dma_start(out=udump.ap(), in_=u[0][:, :]).then_inc(sem["SO"], 16)
            pl.wait_ge(sem["SO"], 32)

        with nc.Block("main") as blk:
            blk.sync(prog_sync)
            blk.scalar(prog_scalar)
            blk.tensor(prog_pe)
            blk.vector(prog_dve)
            blk.gpsimd(prog_pl)

        # After the main block's drain + all-engine barrier, clear every
        # sem except SL (never read) so a re-execution starts clean.
        def prog_cleanup(pl):
            for s in [sem[k] for k in sem if k != "SL"] + SWB + SE:
                pl.sem_clear(s)

        with nc.Block("cleanup") as blk2:
            blk2.gpsimd(prog_cleanup)

    nc.compile()
    return nc


def _get(num_steps: int):
    if num_steps not in _cache:
        _cache[num_steps] = _build(num_steps)
    return _cache[num_steps]


def _shard_inputs(x: np.ndarray, W: np.ndarray):
    bf = ml_dtypes.bfloat16
    xT = np.ascontiguousarray(x[0].reshape(8, 128).T).astype(bf)
    dgv = np.diagonal(W)[D0:].astype(np.float32)
    in_maps = []
    for d in range(NCORES):
        Wd = W[:, CP * d:CP * (d + 1)]
        T = Wd.reshape(KT, 128, CP)
        Wa = np.ascontiguousarray(
            T[0:K0].transpose(1, 0, 2).reshape(128, K0 * CP)
        ).astype(bf)
        # XOR-permuted panels: position block k holds global block d^m(k).
        # m measured on HW (diag): slots 4-7 land with bit1 flipped (D2D
        # slot->engine wiring), so m = [0,1,2,3,6,7,4,5].
        mslot = [0, 1, 2, 3, 6, 7, 4, 5]
        order = np.concatenate(
            [np.arange(8 * (d ^ mslot[k]), 8 * (d ^ mslot[k]) + 8)
             for k in range(8)]
        )
        Tp = T[order]
        Wbk = np.ascontiguousarray(
            Tp.reshape(NBLK, KPB, 128, CP).transpose(0, 2, 1, 3)
            .reshape(NBLK, 128, KPB * CP)
        ).astype(bf)
        Wd4 = W[CP * d:CP * (d + 1), D0:] * dgv[None, :]
        W4 = np.ascontiguousarray(
            Wd4.reshape(8, 128, OUT_N).transpose(1, 0, 2).reshape(128, 8 * OUT_N)
        ).astype(bf)
        in_maps.append({"xT": xT, "Wa": Wa, "Wb": Wbk, "W4": W4})
    return in_maps


def _run(x, W, num_steps, trace=False):
    x = np.asarray(x, dtype=np.float32)
    W = np.asarray(W, dtype=np.float32)
    num_steps = int(num_steps)
    if num_steps == 0:
        return np.zeros(OUT_N, np.float32), None
    if num_steps == 1:
        v1d = W[0:IN_N, D0:].T.astype(np.float64) @ x[0].astype(np.float64)
        return (np.diagonal(W)[D0:] * v1d).astype(np.float32), None
    nc = _get(num_steps)
    in_maps = _shard_inputs(x, W)
    r = run_bass_kernel_spmd(
        nc, in_maps, core_ids=list(range(NCORES)), trace=trace
    )
    outv = np.asarray(r.results[0]["out"], np.float32).T.reshape(OUT_N)
    return outv, r


def kernel(x, W, num_steps) -> np.ndarray:
    outv, _ = _run(x, W, num_steps, trace=False)
    return outv


def run_traced(x, W, num_steps):
    return _run(x, W, num_steps, trace=True)


# revision 22
# speedup vs baseline: 56.3092x; 56.3092x over previous
"""Trainium2 Bass kernel for nn_AdjacencyMatrix — v3: raw engine blocks +
XOR-addressed remote-DMA exchanges (no ncfw collectives).

Math: state_k = W * c_k[:,None] with c_{k+1} = W^T c_k, so the whole
module is num_steps chained matvecs; only the last 256 entries of c_4
(times diag(W)) are needed.  Column-parallel: core r owns W[:, 1024r:
1024(r+1)] (bf16, SBUF-resident) and produces the matching 1024-chunk of
each c_k.

The per-step chunk exchange is done with SBUF->SBUF remote_dma_broadcast
(one slot (0,k) per instruction -> peer tpb = own^k), not ncfw
collectives: this avoids the ~42us CC-stream barrier + ~11us cold-start +
HBM bounce that dominated the collective version.  XOR addressing means
receiver r's u-column block k holds sender r^k's chunk, so the host
permutes each core's SBUF W-panel order to match (position block k =
global k-tile block r^k).  Step 1 (x is only 1024 long) uses a separate
un-permuted 2MB W block; step 4 partials are exchanged the same way and
reduced locally.

All synchronization is manual semaphores (raw nc.Block, no TileContext —
Tile's single-core scheduling sim cannot model remotely-incremented
semaphores).
"""

import contextlib

import ml_dtypes
import numpy as np

import concourse.bass as bass
import concourse.mybir as mybir
from concourse import bacc
from concourse.bass_utils import run_bass_kernel_spmd

N = 8192
IN_N = 1024
OUT_N = 256
NCORES = 8
CP = N // NCORES
KT = N // 128
D0 = N - OUT_N
K0 = 8            # k-tiles in the step-1 (un-permuted) W block
NBLK = 4          # W chase blocks, 16 positions each
KPB = KT // NBLK

F32 = mybir.dt.float32
BF16 = mybir.dt.bfloat16

_cache: dict = {}


def _build(num_steps: int, debug: bool = False):
    assert num_steps >= 2
    n_mid = num_steps - 2
    nc = bacc.Bacc(
        "TRN2", target_bir_lowering=False, debug=debug, num_devices=NCORES
    )
    xT = nc.declare_dram_parameter("xT", [128, 8], BF16, isOutput=False)
    Wa = nc.declare_dram_parameter("Wa", [128, K0 * CP], BF16, isOutput=False)
    Wb = nc.declare_dram_parameter("Wb", [NBLK, 128, KPB * CP], BF16, isOutput=False)
    W4 = nc.declare_dram_parameter("W4", [128, 8 * OUT_N], BF16, isOutput=False)
    outp = nc.declare_dram_parameter("out", [128, 2], F32, isOutput=True)
    udump = nc.declare_dram_parameter("udump", [128, 64], BF16, isOutput=True)
    cc_in = nc.dram_tensor("cc_in", [1, 64], BF16)
    cc_out = nc.dram_tensor("cc_out", [NCORES, 64], BF16, addr_space="Shared")

    es = contextlib.ExitStack()
    with es:
        sem = {}
        for s in ["SX", "SPE", "SD", "SA", "SL", "SPR", "SG", "SO",
                  "SWA", "SW4", "SR", "SC"]:
            sem[s] = es.enter_context(nc.semaphore(s))
        SWB = [es.enter_context(nc.semaphore(f"SWB{b}")) for b in range(NBLK)]
        SE = [es.enter_context(nc.semaphore(f"SE{m}")) for m in range(n_mid + 1)]

        sb = lambda nm, sh, dt: es.enter_context(nc.sbuf_tensor(nm, sh, dt))
        ps = lambda nm, sh, dt: es.enter_context(nc.psum_tensor(nm, sh, dt))

        xt = sb("xt", [128, 8], BF16)
        wa = sb("wa", [128, K0 * CP], BF16)
        wk = sb("wk", [128, KT * CP], BF16)
        w4 = sb("w4", [128, 8 * OUT_N], BF16)
        onef = sb("onef", [1, 1], F32)
        ssb = [sb(f"s{m+1}sb", [1, 1024], F32) for m in range(n_mid + 1)]
        cT = [sb(f"c{m+1}T", [128, 8], BF16) for m in range(n_mid + 1)]
        u = [sb(f"u{m+2}", [128, 64], BF16) for m in range(n_mid)]
        u4 = sb("u4", [128, 8], BF16)
        s4sb = sb("s4sb", [1, OUT_N], F32)
        c4T = sb("c4T", [128, 2], F32)
        acc4 = sb("acc4", [128, 16], F32)
        res = sb("res", [128, 2], F32)
        rtmp = sb("rtmp", [128, 8], F32)

        pA = ps("pA", [128, 512], F32)
        pB = [ps(f"pB{m}", [128, 512], F32) for m in range(n_mid)]
        pD = ps("pD", [128, 512], F32)
        pT = ps("pT", [128, 8], F32)
        pT4 = ps("pT4", [128, 2], F32)

        # ---------------- sync: the W stream ----------------
        def prog_sync(sy):
            sy.dma_start(out=wa[:, :], in_=Wa.ap()).then_inc(sem["SWA"], 16)
            for b in range(NBLK):
                c0 = b * KPB * CP
                sy.dma_start(
                    out=wk[:, c0:c0 + KPB * CP], in_=Wb.ap()[b]
                ).then_inc(SWB[b], 16)
            sy.dma_start(out=w4[:, :], in_=W4.ap()).then_inc(sem["SW4"], 16)

        # ---------------- scalar: xt + step-4 evac ----------------
        def prog_scalar(sc):
            sc.dma_start(out=xt[:, :], in_=xT.ap()).then_inc(sem["SX"], 16)
            sc.wait_ge(sem["SPE"], 3 + 2 * n_mid)
            sc.copy(out=s4sb[0:1, :], in_=pD[0:1, 0:OUT_N]).then_inc(sem["SA"], 1)

        # ---------------- tensor: all matvecs + transposes ----------------
        def waves(pe, pout, usb, wsb, nk, chase=False):
            for p in range(nk):
                if chase and p % KPB == 0:
                    pe.wait_ge(SWB[p // KPB], 16)
                for g in range(4):
                    i = pe.matmul(
                        pout[32 * g:32 * g + 1, 0:256],
                        lhsT=usb[:, p:p + 1],
                        rhs=wsb[:, p * CP + 256 * g:p * CP + 256 * (g + 1)],
                        start=(p == 0),
                        stop=(p == nk - 1),
                        tile_position=(0, 32 * g),
                    )
            return i

        def prog_pe(pe):
            pe.wait_ge(sem["SX"], 16)
            pe.wait_ge(sem["SWA"], 16)
            # step 1 over wa; xt columns are contiguous (stride-1)
            for p in range(K0):
                for g in range(4):
                    i = pe.matmul(
                        pA[32 * g:32 * g + 1, 0:256],
                        lhsT=xt[:, p:p + 1],
                        rhs=wa[:, p * CP + 256 * g:p * CP + 256 * (g + 1)],
                        start=(p == 0),
                        stop=(p == K0 - 1),
                        tile_position=(0, 32 * g),
                    )
            i.then_inc(sem["SPE"], 1)                    # SPE 1: s1 psum done
            spe = 1
            for m in range(n_mid + 1):
                # chunk transposes: s{m+1}sb -> pT (all 4 evac copies done)
                pe.wait_ge(sem["SD"], 5 * m + 4)
                for kl in range(8):
                    i = pe.transpose(
                        pT[0:128, kl:kl + 1],
                        ssb[m][0:1, 128 * kl:128 * (kl + 1)],
                        onef[0:1, 0:1],
                    )
                i.then_inc(sem["SPE"], 1)                # SPE: transposes done
                spe += 1
                if m < n_mid:
                    pe.wait_ge(sem["SG"], m + 1)         # u[m] published
                    i = waves(pe, pB[m], u[m], wk, KT, chase=(m == 0))
                    i.then_inc(sem["SPE"], 1)            # SPE: mid matvec done
                    spe += 1
            # step 4: local tail-column matvec
            pe.wait_ge(sem["SD"], 5 * (n_mid + 1))       # u4 in SBUF
            pe.wait_ge(sem["SW4"], 16)                   # w4 loaded
            for kl in range(8):
                i = pe.matmul(
                    pD[0:1, 0:OUT_N],
                    lhsT=u4[:, kl:kl + 1],
                    rhs=w4[:, OUT_N * kl:OUT_N * (kl + 1)],
                    start=(kl == 0),
                    stop=(kl == 7),
                )
            i.then_inc(sem["SPE"], 1)                    # SPE 3+2n: s4 psum done
            pe.wait_ge(sem["SA"], 1)                     # s4sb evac'd
            for c in range(2):
                i = pe.transpose(
                    pT4[0:128, c:c + 1],
                    s4sb[0:1, 128 * c:128 * (c + 1)],
                    onef[0:1, 0:1],
                )
            i.then_inc(sem["SPE"], 1)                    # SPE 4+2n: pT4 done

        # ---------------- vector: evacs + psum->sbuf copies + reduce ------
        def prog_dve(ve):
            ve.memset(onef[0:1, 0:1], 1.0)
            for m in range(n_mid + 1):
                # evac psum matvec -> s{m+1}sb (4 disjoint copies; DVE
                # completion is out-of-order, so each gets its own inc)
                src_ = pA if m == 0 else pB[m - 1]
                ve.wait_ge(sem["SPE"], 1 + 2 * m)
                for g in range(4):
                    ve.tensor_copy(
                        ssb[m][0:1, 256 * g:256 * (g + 1)],
                        src_[32 * g:32 * g + 1, 0:256],
                    ).then_inc(sem["SD"], 1)
                # chunk copy pT -> c{m+1}T (or u4 for the last)
                ve.wait_ge(sem["SPE"], 2 + 2 * m)
                dst = cT[m] if m < n_mid else u4
                ve.tensor_copy(dst[:, 0:8], pT[0:128, 0:8]).then_inc(sem["SD"], 1)
            ve.wait_ge(sem["SPE"], 4 + 2 * n_mid)
            ve.tensor_copy(c4T[:, 0:2], pT4[0:128, 0:2]).then_inc(sem["SD"], 1)
            # final reduce: res = sum over the 8 slots of acc4 (sem-synced
            # tree; DVE has no intra-engine RAW interlock)
            ve.wait_ge(sem["SG"], n_mid + 1)
            for j in range(4):
                ve.scalar_tensor_tensor(
                    out=rtmp[:, 2 * j:2 * j + 2],
                    in0=acc4[:, 4 * j:4 * j + 2], scalar=0.0,
                    in1=acc4[:, 4 * j + 2:4 * j + 4],
                    op0=mybir.AluOpType.add, op1=mybir.AluOpType.add,
                ).then_inc(sem["SR"], 1)
            ve.wait_ge(sem["SR"], 4)
            for j in range(2):
                ve.scalar_tensor_tensor(
                    out=rtmp[:, 4 * j:4 * j + 2],
                    in0=rtmp[:, 2 * (2 * j):2 * (2 * j) + 2], scalar=0.0,
                    in1=rtmp[:, 2 * (2 * j + 1):2 * (2 * j + 1) + 2],
                    op0=mybir.AluOpType.add, op1=mybir.AluOpType.add,
                ).then_inc(sem["SR"], 1)
            ve.wait_ge(sem["SR"], 6)
            ve.scalar_tensor_tensor(
                out=res[:, 0:2], in0=rtmp[:, 0:2], scalar=0.0,
                in1=rtmp[:, 4:6],
                op0=mybir.AluOpType.add, op1=mybir.AluOpType.add,
            ).then_inc(sem["SD"], 1)

        # ---------------- gpsimd: exchanges + output ----------------
        def prog_pl(pl):
            spr = 0
            for m in range(n_mid + 1):
                last = m == n_mid
                src = c4T if last else cT[m]
                nbytes = 2 if last else 8
                dst = acc4 if last else u[m]
                pl.wait_ge(sem["SD"], 5 * (n_mid + 1) + 1 if last else 5 * m + 5)
                for k in range(1, NCORES):
                    rdests = [None] * NCORES
                    rdests[k] = (0, k)
                    pl.remote_dma_broadcast(
                        out_ap=dst[:, nbytes * k:nbytes * (k + 1)],
                        in_ap=src[:, 0:nbytes],
                        remote_sem=SE[m],
                        local_sem=sem["SL"],
                        rdests=rdests,
                    ).then_inc(sem["SPR"], 1)
                spr += 7
                pl.wait_ge(sem["SPR"], spr)
                pl.trigger_dma(count=7)
                pl.wait_ge(SE[m], 14)
                # self chunk; @complete inc publishes the whole u tile
                pl.tensor_copy(dst[:, 0:nbytes], src[:, 0:nbytes]).then_inc(
                    sem["SG"], 1
                )
            pl.wait_ge(sem["SD"], 5 * (n_mid + 1) + 2)
            pl.dma_start(out=outp.ap(), in_=res[:, 0:2]).then_inc(sem["SO"], 16)
            pl.dma_start(out=udump.ap(), in_=u[0][:, :]).then_inc(sem["SO"], 16)
            pl.wait_ge(sem["SO"], 32)
            # trailing dummy collective: forces cc_enabled NEFF load (gang
            # dispatch across the 8 cores); runs only after all SWDGE
            # remote-DMA activity has fully completed.
            pl.collective_compute(
                "AllGather", mybir.AluOpType.bypass,
                replica_groups=[list(range(NCORES))],
                ins=[cc_in.ap()], outs=[cc_out.ap()],
            ).then_inc(sem["SC"], 1)
            pl.wait_ge(sem["SC"], 1)

        with nc.Block("main") as blk:
            blk.sync(prog_sync)
            blk.scalar(prog_scalar)
            blk.tensor(prog_pe)
            blk.vector(prog_dve)
            blk.gpsimd(prog_pl)

        # After the main block's drain + all-engine barrier, clear every
        # sem except SL (never read) so a re-execution starts clean.
        def prog_cleanup(pl):
            for s in [sem[k] for k in sem if k != "SL"] + SWB + SE:
                pl.sem_clear(s)

        with nc.Block("cleanup") as blk2:
            blk2.gpsimd(prog_cleanup)

    nc.compile()
    return nc


def _get(num_steps: int):
    if num_steps not in _cache:
        _cache[num_steps] = _build(num_steps)
    return _cache[num_steps]


def _shard_inputs(x: np.ndarray, W: np.ndarray):
    bf = ml_dtypes.bfloat16
    xT = np.ascontiguousarray(x[0].reshape(8, 128).T).astype(bf)
    dgv = np.diagonal(W)[D0:].astype(np.float32)
    in_maps = []
    for d in range(NCORES):
        Wd = W[:, CP * d:CP * (d + 1)]
        T = Wd.reshape(KT, 128, CP)
        Wa = np.ascontiguousarray(
            T[0:K0].transpose(1, 0, 2).reshape(128, K0 * CP)
        ).astype(bf)
        # XOR-permuted panels: position block k holds global block d^m(k).
        # m measured on HW (diag): slots 4-7 land with bit1 flipped (D2D
        # slot->engine wiring), so m = [0,1,2,3,6,7,4,5].
        mslot = [0, 1, 2, 3, 6, 7, 4, 5]
        order = np.concatenate(
            [np.arange(8 * (d ^ mslot[k]), 8 * (d ^ mslot[k]) + 8)
             for k in range(8)]
        )
        Tp = T[order]
        Wbk = np.ascontiguousarray(
            Tp.reshape(NBLK, KPB, 128, CP).transpose(0, 2, 1, 3)
            .reshape(NBLK, 128, KPB * CP)
        ).astype(bf)
        Wd4 = W[CP * d:CP * (d + 1), D0:] * dgv[None, :]
        W4 = np.ascontiguousarray(
            Wd4.reshape(8, 128, OUT_N).transpose(1, 0, 2).reshape(128, 8 * OUT_N)
        ).astype(bf)
        in_maps.append({"xT": xT, "Wa": Wa, "Wb": Wbk, "W4": W4})
    return in_maps


def _run(x, W, num_steps, trace=False):
    x = np.asarray(x, dtype=np.float32)
    W = np.asarray(W, dtype=np.float32)
    num_steps = int(num_steps)
    if num_steps == 0:
        return np.zeros(OUT_N, np.float32), None
    if num_steps == 1:
        v1d = W[0:IN_N, D0:].T.astype(np.float64) @ x[0].astype(np.float64)
        return (np.diagonal(W)[D0:] * v1d).astype(np.float32), None
    nc = _get(num_steps)
    in_maps = _shard_inputs(x, W)
    r = run_bass_kernel_spmd(
        nc, in_maps, core_ids=list(range(NCORES)), trace=trace
    )
    outv = np.asarray(r.results[0]["out"], np.float32).T.reshape(OUT_N)
    return outv, r


def kernel(x, W, num_steps) -> np.ndarray:
    outv, _ = _run(x, W, num_steps, trace=False)
    return outv


def run_traced(x, W, num_steps):
    return _run(x, W, num_steps, trace=True)


# revision 23
# speedup vs baseline: 104.5123x; 1.8560x over previous
"""Trainium2 Bass kernel for nn_AdjacencyMatrix — whole-AllGather variant.

Same column-parallel design as kernel.py, but each middle-step exchange is
ONE 2KB AllGather instead of two 1KB halves: Tile's conservative
collective-completion thresholds serialize half A behind half B anyway, so
halving only adds ~7us of serial CC-stream time.  Keeps the 4-block W
stream (HWDGE completion-sem lane fix).
"""

import ml_dtypes
import numpy as np

import concourse.bass as bass
import concourse.mybir as mybir
from concourse import bacc, tile
from concourse.bass_utils import run_bass_kernel_spmd

N = 8192
IN_N = 1024
OUT_N = 256
NCORES = 8
CP = N // NCORES
KT = N // 128
D0 = N - OUT_N
SEG = OUT_N // NCORES
NBLK = 4
KPB = KT // NBLK

F32 = mybir.dt.float32
BF16 = mybir.dt.bfloat16
RG = [list(range(NCORES))]

_cache: dict = {}


def _matvec_waves(nc, pout, u_sb, w_sb, nk, k0_tile=0, ucol0=0):
    for k in range(nk):
        wbase = (k0_tile + k) * CP
        for g in range(4):
            nc.tensor.matmul(
                pout[32 * g:32 * g + 1, 0:256],
                lhsT=u_sb[:, ucol0 + k:ucol0 + k + 1],
                rhs=w_sb[:, wbase + 256 * g:wbase + 256 * (g + 1)],
                start=(k == 0),
                stop=(k == nk - 1),
                tile_position=(0, 32 * g),
            )


def _evac(nc, s_out, pin):
    for g in range(4):
        eng = nc.vector.tensor_copy if g % 2 == 0 else nc.scalar.copy
        eng(out=s_out[0:1, 256 * g:256 * (g + 1)],
            in_=pin[32 * g:32 * g + 1, 0:256])


def _build(num_steps: int):
    assert num_steps >= 2
    n_mid = num_steps - 2
    nc = bacc.Bacc(
        "TRN2", target_bir_lowering=False, debug=False, num_devices=NCORES
    )
    xT = nc.declare_dram_parameter("xT", [128, 8], BF16, isOutput=False)
    Wb = nc.declare_dram_parameter("Wb", [NBLK, 128, KPB * CP], BF16, isOutput=False)
    W4 = nc.declare_dram_parameter("W4", [128, 8 * OUT_N], BF16, isOutput=False)
    ident = nc.declare_dram_parameter("ident", [128, 128], BF16, isOutput=False)
    out = nc.declare_dram_parameter("out", [1, SEG], F32, isOutput=True)

    cc_ins = [
        nc.dram_tensor(f"cc{m}_in", [1, 1024], BF16) for m in range(n_mid + 1)
    ]
    gaths = [
        nc.dram_tensor(f"G{m}", [64, 128], BF16, addr_space="Shared")
        for m in range(n_mid)
    ]
    cc4_in = nc.dram_tensor("cc4_in", [1, OUT_N], F32)
    cc4_out = nc.dram_tensor("cc4_out", [NCORES, SEG], F32)

    with tile.TileContext(nc) as tc:
        with (
            tc.tile_pool(name="small", bufs=1) as small,
            tc.tile_pool(name="wres", bufs=1) as wres,
            tc.tile_pool(name="ppool", bufs=1, space="PSUM") as ppool,
        ):
            xt = small.tile([128, 8], BF16, name="xt")
            nc.scalar.dma_start(out=xt[:, :], in_=xT.ap())
            w4 = small.tile([128, 8 * OUT_N], BF16, name="w4")
            nc.scalar.dma_start(out=w4[:, :], in_=W4.ap())
            idt = small.tile([128, 128], BF16, name="idt")
            nc.scalar.dma_start(out=idt[:, :], in_=ident.ap())
            ones8 = small.tile([8, 1], F32, name="ones8")
            nc.vector.memset(ones8[0:8, :], 1.0)

            wk = wres.tile([128, KT * CP], BF16, name="wk")
            for b in range(NBLK):
                nc.sync.dma_start(
                    out=wk[:, b * KPB * CP:(b + 1) * KPB * CP],
                    in_=Wb.ap()[b],
                )

            pA = ppool.tile([128, 512], F32, name="pA")
            pB = [ppool.tile([128, 512], F32, name=f"pB{m}") for m in range(n_mid)]
            pD = ppool.tile([128, 512], F32, name="pD")
            pv = ppool.tile([1, SEG], F32, name="pv")
            pT = ppool.tile([128, 16], BF16, name="pT")

            _matvec_waves(nc, pA, xt, wk, nk=8, k0_tile=0, ucol0=0)
            s_cur = small.tile([1, 1024], BF16, name="s1")
            _evac(nc, s_cur, pA)

            for m in range(n_mid):
                nc.scalar.dma_start(out=cc_ins[m].ap(), in_=s_cur[0:1, :])
                nc.gpsimd.collective_compute(
                    "AllGather", mybir.AluOpType.bypass, replica_groups=RG,
                    ins=[cc_ins[m].ap()], outs=[gaths[m].ap()],
                )
                u_sb = small.tile([128, KT], BF16, name=f"u{m + 2}")
                nc.scalar.dma_start(out=u_sb[:, :], in_=gaths[m].ap(), transpose=True)
                _matvec_waves(nc, pB[m], u_sb, wk, nk=KT)
                s_cur = small.tile([1, 1024], BF16, name=f"s{m + 2}")
                _evac(nc, s_cur, pB[m])

            u4 = small.tile([128, 16], BF16, name="u4")
            for kl in range(8):
                nc.tensor.transpose(
                    pT[0:128, 2 * kl:2 * kl + 1],
                    s_cur[0:1, 128 * kl:128 * (kl + 1)],
                    idt[0:1, 0:1],
                )
            nc.vector.tensor_copy(u4[:, :], pT[0:128, 0:16])
            for kl in range(8):
                nc.tensor.matmul(
                    pD[0:1, 0:OUT_N],
                    lhsT=u4[:, 2 * kl:2 * kl + 1],
                    rhs=w4[:, OUT_N * kl:OUT_N * (kl + 1)],
                    start=(kl == 0),
                    stop=(kl == 7),
                )
            s4 = small.tile([1, OUT_N], F32, name="s4")
            nc.scalar.copy(out=s4[0:1, :], in_=pD[0:1, 0:OUT_N])
            nc.scalar.dma_start(out=cc4_in.ap(), in_=s4[0:1, :])
            nc.gpsimd.collective_compute(
                "AllToAll", mybir.AluOpType.bypass, replica_groups=RG,
                ins=[cc4_in.ap()], outs=[cc4_out.ap()],
            )
            acc4 = small.tile([NCORES, SEG], F32, name="acc4")
            nc.scalar.dma_start(out=acc4[0:NCORES, :], in_=cc4_out.ap())
            nc.tensor.matmul(
                pv[0:1, :],
                lhsT=ones8[0:NCORES, 0:1],
                rhs=acc4[0:NCORES, :],
                start=True,
                stop=True,
            )
            res = small.tile([1, SEG], F32, name="res")
            nc.vector.tensor_copy(res[0:1, :], pv[0:1, :])
            nc.scalar.dma_start(out=out.ap(), in_=res[0:1, :])

    nc.compile()
    return nc


def _get(num_steps: int):
    if num_steps not in _cache:
        _cache[num_steps] = _build(num_steps)
    return _cache[num_steps]


def _shard_inputs(x: np.ndarray, W: np.ndarray):
    bf = ml_dtypes.bfloat16
    xT = np.ascontiguousarray(x[0].reshape(8, 128).T).astype(bf)
    dgv = np.diagonal(W)[D0:].astype(np.float32)
    idn = np.eye(128, dtype=np.float32).astype(bf)
    in_maps = []
    for d in range(NCORES):
        Wd = W[:, CP * d:CP * (d + 1)]
        Wb = np.ascontiguousarray(
            Wd.reshape(NBLK, KPB, 128, CP).transpose(0, 2, 1, 3)
            .reshape(NBLK, 128, KPB * CP)
        ).astype(bf)
        Wd4 = W[CP * d:CP * (d + 1), D0:] * dgv[None, :]
        W4 = np.ascontiguousarray(
            Wd4.reshape(8, 128, OUT_N).transpose(1, 0, 2).reshape(128, 8 * OUT_N)
        ).astype(bf)
        in_maps.append({"xT": xT, "Wb": Wb, "W4": W4, "ident": idn})
    return in_maps


def _run(x, W, num_steps, trace=False):
    x = np.asarray(x, dtype=np.float32)
    W = np.asarray(W, dtype=np.float32)
    num_steps = int(num_steps)
    if num_steps == 0:
        return np.zeros(OUT_N, np.float32), None
    if num_steps == 1:
        v1d = W[0:IN_N, D0:].T.astype(np.float64) @ x[0].astype(np.float64)
        return (np.diagonal(W)[D0:] * v1d).astype(np.float32), None
    nc = _get(num_steps)
    in_maps = _shard_inputs(x, W)
    r = run_bass_kernel_spmd(
        nc, in_maps, core_ids=list(range(NCORES)), trace=trace
    )
    outv = np.concatenate(
        [np.asarray(r.results[d]["out"], np.float32).reshape(SEG)
         for d in range(NCORES)]
    )
    return outv, r


def kernel(x, W, num_steps) -> np.ndarray:
    outv, _ = _run(x, W, num_steps, trace=False)
    return outv


def run_traced(x, W, num_steps):
    return _run(x, W, num_steps, trace=True)

